# revision 31
# baseline (speedup 1.0000x reference)
"""Trainium2 Bass kernel for fp8-quantized dense matmul (dense_mlp).

Reference computation (per-tensor dynamic fp8 e4m3fn quantization):
    x:     [8, 8192, 512] f32  -> x2d [M=65536, K=512]
    w:     [512, 512] f32
    xs     = 448 / max(amax(|x|), 1e-12);  x_q = e4m3fn(x * xs)
    ws     = 448 / max(amax(|w|), 1e-12);  w_q = e4m3fn(w * ws)
    out    = (x_q @ w_q) * (1/xs) * (1/ws)          [M, 512] f32

Sharding: data-parallel over M across 8 cores (8192 rows each), weight
replicated; the x amax needs a cross-core max-combine.

TRN2 fp8e4 (float8_e4m3) maxes out at +-240 (values in (240, 448] that OCP
e4m3fn can represent are Inf/NaN on TRN). We therefore quantize on-device
with scale' = 224/amax = (448/amax)/2. Scaling by an exact power of two
keeps every quantized value on the same relative grid (q' = q/2 exactly,
modulo the subnormal tail which is negligible), and the dequant factor
computed from the halved scales is exactly 4x the reference's factor,
cancelling the psum/4 -- so the result matches the reference bit-for-bit
up to f32 summation order (HW rel err 4e-7 in Normal matmul mode; the
default DoubleRow fp8 perf mode measures ~1e-4 from the PE's paired-
product accumulation precision, and is ~16% faster end-to-end).

Structure (the kernel is HBM-bound: 16 MiB x in + 16 MiB out + 1 MiB w per
core at ~360 GB/s; everything else must hide under the two DMA windows):

  - The weight path runs FIRST: w DMA ahead of the x chunks, then the
    whole w-scale chain (amax -> partition_all_reduce -> recip -> wq)
    completes ~10 us in, entirely under the x-load window. gpsimd is free
    for the partition reduce/broadcast because there is no firmware
    collective anymore (below), which also removes the old DRAM-bounce
    broadcast of the scale pair.
  - The cross-core amax max-combine is 7 XOR-relative remote_dma_broadcast
    sends (one per peer delta d=1..7, each landing in the receiver's inbox
    column d -- XOR is an involution, so sender delta == receiver column
    with no per-core addressing) instead of a ~10 us (28 us in the cost
    model) firmware AllReduce. Descriptors are generated on the SWDGE ring
    during phase A; the Pool-engine program order (partition_all_reduce ->
    trigger_dma -> monotonic-sem wait -> inbox reduce) provides exactly the
    ordering the fire-time SBUF read needs. Each of the 7 sends bumps the
    receiver's monotonic sem by 2 -> wait target +14 per iteration (the
    monotonic register target keeps this valid inside For_i timing loops).
"""

from contextlib import nullcontext

import numpy as np

import concourse.bacc as bacc
import concourse.bass_isa as bass_isa
import concourse.mybir as mybir
import concourse.tile as tile
from concourse.bass_utils import run_bass_kernel_spmd
from concourse.masks import make_identity

F32 = mybir.dt.float32
FP8 = mybir.dt.float8e4

K = 512
N = 512
KB = K // 128  # k-blocks of 128 (partition-dim contraction tiles)
N_CORES = 8

# fp8 scale ceiling on TRN (e4m3 max normal is 240; 224 = 448/2 keeps the
# quantization grid exactly aligned with the reference's e4m3fn grid)
FP8_CEIL = 224.0


def build_nc(m_shard: int, n_cores: int = N_CORES, use_doublerow: bool = True,
             dma_chunk: int = 4, store_chunk: int = 2, repeat: int = 1,
             phase_a_only: bool = False, ostage_bufs: int | None = None,
             validate_race: bool = False, exchange_in_loop: bool = True):
    """Build + compile the per-core SPMD program.

    m_shard: rows of x handled by this core (must be divisible by 128*dma_chunk)
    repeat: >1 builds a TIMING variant -- the x pipeline (phases A+B, the
        scale chain AND the remote amax exchange) runs in a hardware For_i
        loop `repeat` times so per-iteration time can be resolved above the
        ~0.5ms axon dispatch noise. Only the weight path sits outside the
        loop (it runs once and is off the critical path anyway).
    """
    MT = m_shard // 128          # number of 128-row m-tiles
    CH = MT // dma_chunk         # number of DMA chunks
    SC = MT // store_chunk       # number of store chunks

    nc = bacc.Bacc(
        trn_type="TRN2",
        target_bir_lowering=False,
        debug=False,
        num_devices=n_cores,
    )

    x_in = nc.dram_tensor("x", [m_shard, K], F32, kind="ExternalInput")
    w_in = nc.dram_tensor("w", [K, N], F32, kind="ExternalInput")
    out_d = nc.dram_tensor("out", [m_shard, N], F32, kind="ExternalOutput")

    # DRAM views:
    #  x rows (c*dma_chunk + j)*128 + p  ->  [c, p, j, k]
    x_re = x_in.ap().rearrange("(c j p) k -> c p j k", j=dma_chunk, p=128)
    #  w rows kb*128 + p -> [p, kb, n]
    w_re = w_in.ap().rearrange("(kb p) n -> p kb n", p=128)
    #  out rows t*128 + p -> [p, t, n]  (per-m-tile view; store groups vary)
    out_re = out_d.ap().rearrange("(t p) n -> p t n", p=128)

    with tile.TileContext(nc) as tc:
        if not validate_race:
            # The rdma descriptor preps read the send buffer at TRIGGER time
            # (Pool program order: preps -> partition_all_reduce -> trigger),
            # but the race detector attributes the read to the prep, flagging
            # the later amax write as a WAR race. The ordering is sound on
            # HW (gpsimd ops hold the Pool sequencer until their SBUF write
            # committed; the SDMA source read starts at the doorbell). A
            # validate_race build keeps the detector on by generating the
            # descriptors after the amax write instead (slower, same
            # semantics) -- used in testing to vet everything else.
            tc.race_detector_enabled = False
        with (
            tc.tile_pool(name="pers", bufs=1) as pers,
            tc.tile_pool(name="xld", bufs=max(2, 16 // dma_chunk)) as xld,
            tc.tile_pool(name="xqp", bufs=8) as xqp,
            tc.tile_pool(
                name="ostage",
                bufs=ostage_bufs if ostage_bufs is not None
                else (4 if store_chunk <= 2 else 3),
            ) as ostage,
            tc.tile_pool(name="tpsum", bufs=2, space="PSUM") as tpsum,
            tc.tile_pool(name="opsum", bufs=2, space="PSUM") as opsum,
        ):
            # ---------------- persistent tiles ----------------
            ident = pers.tile([128, 128], F32)
            w_f32 = pers.tile([128, KB, N], F32)
            wq = pers.tile([128, KB, N], FP8)
            xt_f32 = pers.tile([128, KB, m_shard], F32)   # transposed x (K on partitions)
            amax_slots = pers.tile([128, 2 * CH + 1], F32)
            inbox = pers.tile([128, 8], F32)  # col d <- peer (me XOR d), d=1..7

            def sc(name):
                return pers.tile([128, 1], F32, name=name)

            wa_part, wa_bc, wa_c, wa_r, wsc, wsc_inv = (
                sc("wa_part"), sc("wa_bc"), sc("wa_c"), sc("wa_r"),
                sc("wsc"), sc("wsc_inv"))
            xa_part, xa_g, xa_c, xa_r = (
                sc("xa_part"), sc("xa_g"), sc("xa_c"), sc("xa_r"))
            xsc, inv_xsc, dsc = sc("xsc"), sc("inv_xsc"), sc("dsc")

            make_identity(nc, ident)
            # defined producer for the rdma preps' fire-time read; col 0 is
            # the local amax (partition_all_reduce writes it there directly,
            # which keeps the post-exchange merge to a single reduce)
            nc.vector.memset(inbox, 0.0)

            def weight_path():
                # Off the critical path: w loads via the SWDGE (gpsimd)
                # queue so the x chunks own the sync-engine DMA queue from
                # t=0, then the whole scale chain finishes ~10us in.
                # partition_all_reduce gives an exact all-partition
                # broadcast of the global |w| max directly (each partition
                # then computes bit-identical scales from identical
                # inputs), so no PE-transpose / DRAM-bounce is needed.
                nc.gpsimd.dma_start(out=w_f32[:], in_=w_re)
                nc.vector.tensor_reduce(
                    out=wa_part[:], in_=w_f32[:], axis=mybir.AxisListType.XY,
                    op=mybir.AluOpType.max, apply_absolute_value=True,
                )
                nc.gpsimd.partition_all_reduce(
                    wa_bc[:], wa_part[:], channels=128,
                    reduce_op=bass_isa.ReduceOp.max,
                )
                nc.vector.tensor_scalar_max(wa_c[:], wa_bc[:], 1e-12)
                # wsc = 224 * (1/wa)  (TT divide is not a valid TRN2 DVE op;
                # the extra rounding vs fl(224/wa) is <=1ulp on the scale)
                nc.vector.reciprocal(wa_r[:], wa_c[:])
                nc.vector.tensor_scalar_mul(wsc[:], wa_r[:], FP8_CEIL)
                nc.vector.reciprocal(wsc_inv[:], wsc[:])
                # quantize weight: wq = fp8(w * wsc)
                nc.scalar.mul(wq[:], w_f32[:], wsc[:, 0:1])

            weight_path()

            rsem = nc.alloc_semaphore("rdma_rsem") if n_cores > 1 else None
            lsem = nc.alloc_semaphore("rdma_lsem") if n_cores > 1 else None
            rwait_reg = None
            if n_cores > 1 and repeat > 1:
                # accumulating wait target (+14/iteration) so the exchange
                # stays valid inside the For_i timing loop
                rwait_reg = nc.vector.alloc_register("rsem_target")
                nc.vector.reg_mov(rwait_reg, 0)

            loop_cm = (
                tc.For_i(0, repeat, 1, hint_engines=(mybir.EngineType.PE,))
                if repeat > 1 else nullcontext()
            )
            with loop_cm:
                x_pipeline(
                    nc, tc, CH, dma_chunk, SC, store_chunk, KB, MT, N,
                    x_re, out_re, xld, xqp, ostage, tpsum, opsum,
                    ident, w_f32, wq, xt_f32, amax_slots, inbox,
                    xa_part, xa_g, xa_c, xa_r,
                    xsc, inv_xsc, wsc_inv, dsc,
                    n_cores if (exchange_in_loop or repeat == 1) else 1,
                    use_doublerow, rsem, lsem, rwait_reg,
                    phase_a_only=phase_a_only, preps_late=validate_race,
                )

    nc.compile()
    return nc


def x_pipeline(nc, tc, CH, dma_chunk, SC, store_chunk, KB, MT, N,
               x_re, out_re, xld, xqp, ostage, tpsum, opsum,
               ident, w_f32, wq, xt_f32, amax_slots, inbox,
               xa_part, xa_g, xa_c, xa_r,
               xsc, inv_xsc, wsc_inv, dsc,
               n_cores, use_doublerow, rsem, lsem, rwait_reg,
               phase_a_only=False, preps_late=False):
            # -------- remote amax exchange: prep descriptors early --------
            # Desc-gen (~1us each on the Pool sequencer) runs during phase A;
            # the payload SBUF read happens at trigger time, after
            # partition_all_reduce has written xa_bc (Pool executes in
            # order). Send delta d lands in the receiver's inbox column d.
            def emit_preps():
                for d in range(1, 8):
                    rdests = [None] * 8
                    rdests[d] = (0, d)  # XOR-relative, same-chip
                    nc.gpsimd.remote_dma_broadcast(
                        out_ap=inbox[:, d:d + 1], in_ap=inbox[:, 0:1],
                        remote_sem=rsem, local_sem=lsem,
                        rdests=rdests,
                    )

            if n_cores > 1 and not preps_late:
                emit_preps()

            # ---------------- phase A: load x, amax, transpose ----------------
            # DVE reduces run at ~1 elem/lane/cycle (no 2x mode), so a
            # full-chunk abs-max (~2.2us) after the last byte would pace the
            # global barrier. The last chunk's loads are split [half, 1, 1]
            # m-tiles so the final reduce covers a single m-tile right after
            # the (small) final load's completion receipt.
            half = dma_chunk // 2
            slot = 0
            for c in range(CH):
                xt = xld.tile([128, dma_chunk, K], F32)
                if c < CH - 1:
                    nc.sync.dma_start(out=xt[:], in_=x_re[c])
                    nc.vector.tensor_reduce(
                        out=amax_slots[:, slot:slot + 1], in_=xt[:],
                        axis=mybir.AxisListType.XY,
                        op=mybir.AluOpType.max, apply_absolute_value=True,
                    )
                    slot += 1
                else:
                    pieces = [(0, half)] + [(j, 1) for j in range(half, dma_chunk)]
                    for j0, nj in pieces:
                        nc.sync.dma_start(
                            out=xt[:, j0:j0 + nj, :], in_=x_re[c, :, j0:j0 + nj, :])
                        nc.vector.tensor_reduce(
                            out=amax_slots[:, slot:slot + 1],
                            in_=xt[:, j0:j0 + nj, :],
                            axis=mybir.AxisListType.XY,
                            op=mybir.AluOpType.max, apply_absolute_value=True,
                        )
                        slot += 1
                for j2 in range(dma_chunk // 2):
                    # two m-tiles per PSUM tile (2 banks) -> one FD-1024 evac
                    tp = tpsum.tile([128, 2, KB, 128], F32)
                    for j in (2 * j2, 2 * j2 + 1):
                        for kb in range(KB):
                            nc.tensor.transpose(
                                tp[:, j % 2, kb, :],
                                xt[:, j, kb * 128:(kb + 1) * 128], ident[:],
                            )
                    i = c * dma_chunk + 2 * j2   # first of the 2 m-tiles
                    # evacuate transposed f32 tiles (ACT; PSUM -> SBUF)
                    # dest [128, kb, 2, 128] viewed per kb: [2 m-tiles, 128]
                    nc.scalar.copy(
                        out=xt_f32[:, :, i * 128:(i + 2) * 128]
                        .rearrange("p kb (j m) -> p j kb m", j=2),
                        in_=tp[:],
                    )

            if phase_a_only:
                return

            # ---------------- amax finalize + cross-core max ----------------
            nc.vector.tensor_reduce(
                out=xa_part[:], in_=amax_slots[:, :slot],
                axis=mybir.AxisListType.X, op=mybir.AluOpType.max,
            )
            nc.gpsimd.partition_all_reduce(
                inbox[:, 0:1], xa_part[:], channels=128,
                reduce_op=bass_isa.ReduceOp.max,
            )
            if n_cores > 1:
                if preps_late:
                    emit_preps()                 # race-clean validation build
                # signals_writable pins the trigger AFTER the
                # partition_all_reduce write of inbox[:,0] (tile would
                # otherwise reorder it up to its only edge, the preps, and
                # the sends would fire with the memset zeros)
                nc.gpsimd.trigger_dma(
                    None, signals_writable=[inbox[:, 0:1]],
                )                                # fire the 7 prepared sends
                # The remote-sem wait must be invisible to tile's scheduling
                # sim (no peers there -> deadlock), so the wait AND the op
                # reading the remotely-written inbox live in a critical
                # section; downstream ops depend on xa_g, which the section
                # writes, so tile orders them after its exit drain.
                with tc.tile_critical(name="amax_xchg"):
                    # 7 peers x 2 incs each; gates the in-order DVE stream
                    if rwait_reg is not None:
                        nc.vector.reg_add(rwait_reg, rwait_reg, 14)
                        nc.vector.wait_ge(rsem, rwait_reg)
                    else:
                        nc.vector.wait_ge(rsem, 14)
                    nc.vector.tensor_reduce(
                        out=xa_g[:], in_=inbox[:, 0:8],
                        axis=mybir.AxisListType.X, op=mybir.AluOpType.max,
                    )
            else:
                nc.vector.tensor_copy(xa_g[:], inbox[:, 0:1])

            nc.vector.tensor_scalar_max(xa_c[:], xa_g[:], 1e-12)
            nc.vector.reciprocal(xa_r[:], xa_c[:])
            nc.vector.tensor_scalar_mul(xsc[:], xa_r[:], FP8_CEIL)
            def emit_dsc():
                # emitted after the first quantize: DVE executes in order, so
                # placing these two ops between xsc and quantize_0 would delay
                # the first matmul/store; dsc is only needed by the first
                # dequant, which waits on the matmuls anyway
                nc.vector.reciprocal(inv_xsc[:], xsc[:])
                nc.vector.tensor_tensor(
                    out=dsc[:], in0=inv_xsc[:], in1=wsc_inv[:, 0:1],
                    op=mybir.AluOpType.mult,
                )

            # ---------------- phase B: quantize, matmul, dequant, store -------
            # One group = quantize -> matmul -> dequant -> store. Groups are
            # 2 m-tiles (one 2-bank PSUM tile) except the FIRST and LAST,
            # which are 1 m-tile: the first shortens the latency from xsc to
            # the first out DMA, the last shrinks the final store whose
            # completion receipt sits in the kernel's exit barrier.
            assert MT % 2 == 0
            groups = [(0, 1)] + [(1 + 2 * i, 2) for i in range((MT - 2) // 2)] \
                + [(MT - 1, 1)]
            for m0, nt in groups:
                ob = ostage.tile([128, 2, N], F32)
                po = opsum.tile([128, 2, N], F32)
                # quantize up to 2 m-tiles per DVE op (2x fp32 SBUF mode)
                xq_t2 = xqp.tile([128, KB, 2 * 128], FP8)
                nc.vector.tensor_scalar_mul(
                    xq_t2[:, :, :nt * 128],
                    xt_f32[:, :, m0 * 128:(m0 + nt) * 128], xsc[:],
                )
                if emit_dsc is not None:
                    emit_dsc()
                    emit_dsc = None
                for j in range(nt):
                    xq_t = xq_t2[:, :, j * 128:(j + 1) * 128]
                    if use_doublerow:
                        for kb in range(0, KB, 2):
                            nc.tensor.matmul(
                                po[:, j, :], xq_t[:, kb:kb + 2, :],
                                wq[:, kb:kb + 2, :],
                                start=(kb == 0), stop=(kb == KB - 2),
                                perf_mode=mybir.MatmulPerfMode.DoubleRow,
                            )
                    else:
                        for kb in range(KB):
                            nc.tensor.matmul(
                                po[:, j, :], xq_t[:, kb, :], wq[:, kb, :],
                                start=(kb == 0), stop=(kb == KB - 1),
                            )
                # dequant on ACT (activation Copy with scale AP)
                nc.scalar.mul(ob[:, :nt, :], po[:, :nt, :], dsc[:])
                nc.sync.dma_start(out=out_re[:, m0:m0 + nt, :], in_=ob[:, :nt, :])


_CACHE: dict = {}


def _get_compiled(m_shard: int, **kw):
    key = (m_shard, tuple(sorted(kw.items())))
    if key not in _CACHE:
        _CACHE[key] = build_nc(m_shard, **kw)
    return _CACHE[key]


def run(x2d: np.ndarray, w: np.ndarray, trace: bool = False, **build_kw):
    """Run the SPMD kernel on [M, K] x and return ([M, N] out, BassKernelResults)."""
    M = x2d.shape[0]
    assert M % N_CORES == 0
    m_shard = M // N_CORES
    nc = _get_compiled(m_shard, **build_kw)
    shards = x2d.reshape(N_CORES, m_shard, K)
    w = np.ascontiguousarray(w, dtype=np.float32)
    in_maps = [
        {"x": np.ascontiguousarray(shards[c]), "w": w} for c in range(N_CORES)
    ]
    res = run_bass_kernel_spmd(nc, in_maps, core_ids=list(range(N_CORES)),
                               trace=trace)
    out = np.concatenate([res.results[c]["out"] for c in range(N_CORES)], axis=0)
    return out, res


def kernel(x: np.ndarray, weight: np.ndarray) -> np.ndarray:
    x = np.asarray(x, dtype=np.float32)
    weight = np.asarray(weight, dtype=np.float32)
    B, S, k = x.shape
    assert k == K
    out, _ = run(x.reshape(-1, K), weight)
    return out.reshape(B, S, N).astype(np.float32)
